# revision 2
# baseline (speedup 1.0000x reference)
"""BiEncoder (bidirectional LSTM over video features) Trainium2 kernel, v2.

Sharding: 8 NeuronCores = 8 batch groups (BC=32 each); every core runs BOTH
directions. The embed (video @ W_e.T) is computed once per core and shared by
the two directions (dir1 reads it time-reversed), halving embed PE work vs a
direction-sharded layout.

Per-core structure:
  embed (per 8-step chunk): v = video @ W_e.T          (fp16 matmul, FD=256)
  scan (64 rounds, 2 dirs):
    pg   = W_ih_d @ v_t + bias_d + W_hh_d @ h_prev     (PSUM accumulation,
           bias enters via a constant one-hot input tile; no DVE add needed)
    s_if = sigmoid(pg[i,f])  t_g = tanh(pg[g])  s_o = sigmoid(pg[o])   (ACT)
    c    = s_f*c + s_i*t_g;  h = s_o*tanh(c)                           (DVE)
  The next round's input-projection matmuls (independent of h) and the
  rationed embed chunks keep the PE busy during each round's pointwise chain.

Embed chunk order (0,7),(1,6),(2,5),(3,4): dir0 consumes chunks forward,
dir1 backward, so both directions can start immediately.
"""

import sys
import time

for _p in ("/opt/trn_rl_repo", "/root/.axon_site/_ro/trn_rl_repo"):
    if _p not in sys.path:
        sys.path.insert(0, _p)

import numpy as np
import jax

try:
    jax.config.update("jax_compilation_cache_dir", "/tmp/jax_cc_cache")
    jax.config.update("jax_persistent_cache_min_entry_size_bytes", 0)
    jax.config.update("jax_persistent_cache_min_compile_time_secs", 0.0)
except Exception:
    pass

import concourse.tile as tile
from concourse import bacc, mybir
from concourse.bass import ts
from concourse.bass_utils import run_bass_kernel_spmd

F16 = mybir.dt.float16
F32 = mybir.dt.float32
AF = mybir.ActivationFunctionType
OP = mybir.AluOpType

B, T, F, P, H = 256, 64, 2048, 512, 512
NB = 8          # batch groups = cores
BC = B // NB    # 32 per-core batch
TC = 8          # timesteps per embed chunk
NCHUNK = T // TC
KF = F // 128   # 16 F tiles
KP = P // 128   # 4  P tiles
KPB = KP + 1    # +1 bias tile (constant one-hot input row)
KH = H // 128   # 4  H tiles
MG = 4 * H // 128  # 16 gate tiles = 4 kinds (i,f,g,o) x KH

# embed chunk emission order: feeds dir0 (forward) and dir1 (backward)
CHUNK_ORDER = [0, 7, 1, 6, 2, 5, 3, 4]


def build_nc():
    nc = bacc.Bacc("TRN2", target_bir_lowering=False, debug=False, num_devices=8)

    vt_d = nc.dram_tensor("vt", [NCHUNK, 128, KF, TC, BC], F16, kind="ExternalInput")
    wet_d = nc.dram_tensor("w_et", [128, KF, P], F16, kind="ExternalInput")
    wih_d = nc.dram_tensor("w_ih", [128, 2, KPB, 4 * H], F16, kind="ExternalInput")
    whh_d = nc.dram_tensor("w_hh", [128, 2, KH, 4 * H], F16, kind="ExternalInput")
    out_d = nc.dram_tensor("out_h", [2, NCHUNK, 128, TC, KH, BC], F16,
                           kind="ExternalOutput")

    with tile.TileContext(nc) as tc:
        with (
            tc.tile_pool(name="const", bufs=1) as const,
            tc.tile_pool(name="vload", bufs=4) as vload,
            tc.tile_pool(name="vsbp", bufs=1) as vsbp,
            tc.tile_pool(name="hst0", bufs=2) as hst0p,
            tc.tile_pool(name="hst1", bufs=2) as hst1p,
            tc.tile_pool(name="sg", bufs=3) as sgp,
            tc.tile_pool(name="cst", bufs=2) as cstp,
            tc.tile_pool(name="tmp", bufs=2) as tmpp,
            tc.tile_pool(name="psv", bufs=2, space="PSUM") as psv,
            tc.tile_pool(name="pg0", bufs=3, space="PSUM") as pg0p,
            tc.tile_pool(name="pg1", bufs=3, space="PSUM") as pg1p,
        ):
            # resident weights; order matters: wet + first video chunks gate
            # the embed prologue, wih/whh only gate round 0
            wet = const.tile([128, KF, P], F16)
            nc.sync.dma_start(wet[:], wet_d.ap())
            wih = const.tile([128, 2, KPB, 4 * H], F16)
            whh = const.tile([128, 2, KH, 4 * H], F16)

            # constant one-hot input column for the bias matmul: partition 0
            # carries 1.0 so (bias-row weight tile) @ onec adds the gate bias
            onec = const.tile([128, BC], F16)
            nc.gpsimd.memset(onec[:], 0.0)
            nc.gpsimd.memset(onec[0:1, :], 1.0)

            # full embed output, shared by both directions
            vsb = vsbp.tile([128, KP, T, BC], F16)

            pgp = (pg0p, pg1p)

            vch_tiles = {}

            def dma_chunk(c):
                vch = vload.tile([128, KF, TC * BC], F16, tag="vch")
                nc.sync.dma_start(vch[:], vt_d.ap()[c].rearrange("p ko t b -> p ko (t b)"))
                vch_tiles[c] = vch

            def embed_item(c, mp):
                vch = vch_tiles[c]
                pv = psv.tile([128, TC * BC], F32, tag="pv")
                for ko in range(KF):
                    nc.tensor.matmul(
                        pv[:],
                        wet[:, ko, ts(mp, 128)],
                        vch[:, ko, :],
                        start=(ko == 0),
                        stop=(ko == KF - 1),
                    )
                # psum -> sbuf copy on DVE (ACT is the scarcer engine here)
                nc.vector.tensor_scalar_mul(
                    vsb[:, mp, c * TC : (c + 1) * TC, :].rearrange("p t b -> p (t b)"),
                    pv[:],
                    1.0,
                )

            def xg_mms(d, t):
                """Input-projection + bias matmuls for dir d, step t (no h
                dependence). The pg tile is one PSUM bank = one zero region:
                start=True only on the very first matmul into the tile (it
                zeroes the whole region), stop=True only on the very last
                (end of hh, or end of xg at t==0)."""
                v_idx = t if d == 0 else T - 1 - t
                pg = pgp[d].tile([128, 4, KH, BC], F32, tag=f"pg{d}")
                last = t == 0  # step 0 has no hh matmuls: close the group now
                for m in range(MG):
                    kind, kh = divmod(m, KH)
                    for kp in range(KPB):
                        rhs = onec[:] if kp == KP else vsb[:, kp, v_idx, :]
                        nc.tensor.matmul(
                            pg[:, kind, kh, :],
                            wih[:, d, kp, ts(m, 128)],
                            rhs,
                            start=(m == 0 and kp == 0),
                            stop=(last and m == MG - 1 and kp == KPB - 1),
                            skip_group_check=True,
                        )
                return pg

            def hh_mms(d, t, pg, h_prev):
                for m in range(MG):
                    kind, kh = divmod(m, KH)
                    for khi in range(KH):
                        nc.tensor.matmul(
                            pg[:, kind, kh, :],
                            whh[:, d, khi, ts(m, 128)],
                            h_prev[:, khi, :],
                            start=False,
                            stop=(m == MG - 1 and khi == KH - 1),
                            skip_group_check=True,
                        )

            def act_gates(d, t, pg):
                sg = sgp.tile([128, 4, KH, BC], F16, tag=f"sg{d}")
                nc.scalar.activation(sg[:, 0:2, :, :], pg[:, 0:2, :, :], AF.Sigmoid)
                nc.scalar.activation(sg[:, 2, :, :], pg[:, 2, :, :], AF.Tanh)
                nc.scalar.activation(sg[:, 3, :, :], pg[:, 3, :, :], AF.Sigmoid)
                return sg

            def dve_c(d, t, sg, c_prev):
                c_new = cstp.tile([128, KH, BC], F32, tag=f"c{d}")
                if t == 0:
                    nc.vector.tensor_tensor(c_new[:], sg[:, 0, :, :], sg[:, 2, :, :],
                                            OP.mult)
                else:
                    m2 = tmpp.tile([128, KH, BC], F16, tag=f"m2{d}")
                    nc.vector.tensor_tensor(m2[:], sg[:, 0, :, :], sg[:, 2, :, :],
                                            OP.mult)
                    m1 = tmpp.tile([128, KH, BC], F32, tag=f"m1{d}")
                    nc.vector.tensor_tensor(m1[:], sg[:, 1, :, :], c_prev[:], OP.mult)
                    nc.vector.tensor_tensor(c_new[:], m1[:], m2[:], OP.add)
                return c_new

            def act_tanh_c(d, c_new):
                tcn = tmpp.tile([128, KH, BC], F16, tag=f"tc{d}")
                nc.scalar.activation(tcn[:], c_new[:], AF.Tanh)
                return tcn

            def dve_h(d, t, sg, tcn, hst):
                h_new = hst[:, t % TC, :, :]
                nc.vector.tensor_tensor(h_new, sg[:, 3, :, :], tcn[:], OP.mult)
                return h_new

            # ---- embed work-list construction -------------------------------
            # pair p = chunks CHUNK_ORDER[2p], CHUNK_ORDER[2p+1]; pair 0 runs
            # in the prologue; pair p>=1 must complete before round 8p-1 ends
            # (xg(8p) consumes it), rationed over rounds 8(p-1)..8p-2.
            embed_sched = {r: [] for r in range(T)}
            for p in range(1, 4):
                ca, cb = CHUNK_ORDER[2 * p], CHUNK_ORDER[2 * p + 1]
                items = [(ca, mp) for mp in range(KP)] + [(cb, mp) for mp in range(KP)]
                r0 = 8 * (p - 1)
                # DMA the pair's chunks right away at the window start
                embed_sched[r0].append(("dma", ca))
                embed_sched[r0].append(("dma", cb))
                for j, it in enumerate(items):
                    embed_sched[r0 + j % 7].append(("embed",) + it)

            # ---- prologue ---------------------------------------------------
            dma_chunk(0)
            dma_chunk(7)
            nc.sync.dma_start(wih[:], wih_d.ap())
            nc.sync.dma_start(whh[:], whh_d.ap())
            for mp in range(KP):
                embed_item(0, mp)
            for mp in range(KP):
                embed_item(7, mp)
            pg_cur = [xg_mms(0, 0), xg_mms(1, 0)]

            h_prev = [None, None]
            c_prev = [None, None]
            hst = [None, None]

            # ---- scan rounds ------------------------------------------------
            for t in range(T):
                if t % TC == 0:
                    hst[0] = hst0p.tile([128, TC, KH, BC], F16, tag="hst0", name="hst0")
                    hst[1] = hst1p.tile([128, TC, KH, BC], F16, tag="hst1", name="hst1")

                # PE: finish this round's gates (hh), per dir
                for d in (0, 1):
                    if t > 0:
                        hh_mms(d, t, pg_cur[d], h_prev[d])

                # ACT: gate activations (tanh_c comes later, after DVE c)
                sgs = [act_gates(d, t, pg_cur[d]) for d in (0, 1)]
                # DVE: c chain per dir
                c_news = [dve_c(d, t, sgs[d], c_prev[d]) for d in (0, 1)]
                # ACT: tanh(c)
                tcns = [act_tanh_c(d, c_news[d]) for d in (0, 1)]
                # DVE: h = s_o * tanh(c)
                for d in (0, 1):
                    h_prev[d] = dve_h(d, t, sgs[d], tcns[d], hst[d])
                    c_prev[d] = c_news[d]

                # PE filler: next round's input projections (h-independent)
                if t + 1 < T:
                    pg_cur = [xg_mms(0, t + 1), xg_mms(1, t + 1)]

                # rationed embed work + chunk DMAs
                for item in embed_sched[t]:
                    if item[0] == "dma":
                        dma_chunk(item[1])
                    else:
                        embed_item(item[1], item[2])

                # chunk boundary: ship outputs
                if t % TC == TC - 1:
                    c_out = t // TC
                    nc.sync.dma_start(out_d.ap()[0][c_out], hst[0][:])
                    nc.sync.dma_start(out_d.ap()[1][c_out], hst[1][:])

    nc.compile()
    return nc


_CACHED_NC = None


def _get_nc():
    global _CACHED_NC
    if _CACHED_NC is None:
        _CACHED_NC = build_nc()
    return _CACHED_NC


def _prep_inputs(video_feats, W_e, b_e, W_ih1, W_hh1, b_ih1, b_hh1,
                 W_ih2, W_hh2, b_ih2, b_hh2):
    """Build the 8 per-core input maps (host-side shard + layout prep)."""
    wet = np.ascontiguousarray(
        W_e.T.astype(np.float16).reshape(KF, 128, P).transpose(1, 0, 2)
    )

    wih_all = np.zeros((128, 2, KPB, 4 * H), np.float16)
    whh_all = np.zeros((128, 2, KH, 4 * H), np.float16)
    for d, (W_ih, W_hh, b_ih, b_hh) in enumerate((
        (W_ih1, W_hh1, b_ih1, b_hh1),
        (W_ih2, W_hh2, b_ih2, b_hh2),
    )):
        wih_all[:, d, :KP, :] = (
            W_ih.T.astype(np.float16).reshape(KP, 128, 4 * H).transpose(1, 0, 2)
        )
        # gate bias (incl. embed bias pushed through W_ih) rides contraction
        # row 0 of the extra input tile
        b_full = (b_ih + b_hh + W_ih @ b_e).astype(np.float16)
        wih_all[0, d, KP, :] = b_full
        whh_all[:, d, :, :] = (
            W_hh.T.astype(np.float16).reshape(KH, 128, 4 * H).transpose(1, 0, 2)
        )

    vt_full = np.ascontiguousarray(video_feats.transpose(2, 1, 0)).astype(np.float16)

    in_maps = []
    for core in range(8):
        src = vt_full[:, :, core * BC : (core + 1) * BC]
        vt = np.ascontiguousarray(
            src.reshape(KF, 128, NCHUNK, TC, BC).transpose(2, 1, 0, 3, 4)
        )
        in_maps.append({
            "vt": vt,
            "w_et": wet,
            "w_ih": wih_all,
            "w_hh": whh_all,
        })
    return in_maps


last_exec_ns = None
last_wall_s = None


def kernel(**inputs):
    global last_exec_ns, last_wall_s
    nc = _get_nc()
    inputs = {k: np.asarray(v, dtype=np.float32) for k, v in inputs.items()}
    in_maps = _prep_inputs(**inputs)
    t0 = time.perf_counter()
    res = run_bass_kernel_spmd(nc, in_maps, core_ids=list(range(8)))
    last_wall_s = time.perf_counter() - t0
    last_exec_ns = res.exec_time_ns

    lstm1 = np.empty((B, T, H), np.float32)
    lstm2 = np.empty((B, T, H), np.float32)
    for core in range(8):
        oh = res.results[core]["out_h"]  # [2, NCHUNK, 128, TC, KH, BC] f16
        # [dir, chunk, p, tc, kh, bc] -> [dir, bc, chunk*tc, kh*p]
        h = np.transpose(oh.astype(np.float32), (0, 5, 1, 3, 4, 2)).reshape(
            2, BC, T, H
        )
        lstm1[core * BC : (core + 1) * BC] = h[0]
        lstm2[core * BC : (core + 1) * BC] = h[1][:, ::-1, :]
    return (lstm1, lstm2)


# revision 3
# speedup vs baseline: 1.0035x; 1.0035x over previous
"""BiEncoder (bidirectional LSTM over video features) Trainium2 kernel, v2.

Sharding: 8 NeuronCores = 8 batch groups (BC=32 each); every core runs BOTH
directions. The embed (video @ W_e.T) is computed once per core and shared by
the two directions (dir1 reads it time-reversed), halving embed PE work vs a
direction-sharded layout.

Per-core structure:
  embed (per 8-step chunk): v = video @ W_e.T          (fp16 matmul, FD=256)
  scan (64 rounds, 2 dirs):
    pg   = W_ih_d @ v_t + bias_d + W_hh_d @ h_prev     (PSUM accumulation,
           bias enters via a constant one-hot input tile; no DVE add needed)
    s_if = sigmoid(pg[i,f])  t_g = tanh(pg[g])  s_o = sigmoid(pg[o])   (ACT)
    c    = s_f*c + s_i*t_g;  h = s_o*tanh(c)                           (DVE)
  The next round's input-projection matmuls (independent of h) and the
  rationed embed chunks keep the PE busy during each round's pointwise chain.

Embed chunk order (0,7),(1,6),(2,5),(3,4): dir0 consumes chunks forward,
dir1 backward, so both directions can start immediately.
"""

import sys
import time

for _p in ("/opt/trn_rl_repo", "/root/.axon_site/_ro/trn_rl_repo"):
    if _p not in sys.path:
        sys.path.insert(0, _p)

import numpy as np
import jax

try:
    jax.config.update("jax_compilation_cache_dir", "/tmp/jax_cc_cache")
    jax.config.update("jax_persistent_cache_min_entry_size_bytes", 0)
    jax.config.update("jax_persistent_cache_min_compile_time_secs", 0.0)
except Exception:
    pass

import concourse.tile as tile
from concourse import bacc, mybir
from concourse.bass import ts
from concourse.bass_utils import run_bass_kernel_spmd

F16 = mybir.dt.float16
F32 = mybir.dt.float32
F8 = mybir.dt.float8e4
WS = 16.0
AF = mybir.ActivationFunctionType
OP = mybir.AluOpType

B, T, F, P, H = 256, 64, 2048, 512, 512
NB = 8          # batch groups = cores
BC = B // NB    # 32 per-core batch
TC = 8          # timesteps per embed chunk
NCHUNK = T // TC
KF = F // 128   # 16 F tiles
KP = P // 128   # 4  P tiles
KPB = KP + 1    # +1 bias tile (constant one-hot input row)
KH = H // 128   # 4  H tiles
MG = 4 * H // 128  # 16 gate tiles = 4 kinds (i,f,g,o) x KH

# embed chunk emission order: feeds dir0 (forward) and dir1 (backward)
CHUNK_ORDER = [0, 7, 1, 6, 2, 5, 3, 4]


def build_nc():
    nc = bacc.Bacc("TRN2", target_bir_lowering=False, debug=False, num_devices=8)

    vt_d = nc.dram_tensor("vt", [NCHUNK, 128, KF, TC, BC], F16, kind="ExternalInput")
    # wet split into KP column slices so the first embed item can start after
    # a quarter of the weight DMA
    wet_d = nc.dram_tensor("w_et", [KP, 128, KF, 128], F16, kind="ExternalInput")
    wih_d = nc.dram_tensor("w_ih", [2, 128, KPB, 4 * H], F16, kind="ExternalInput")
    whh_d = nc.dram_tensor("w_hh", [2, 128, KH, 4 * H], F8, kind="ExternalInput")
    out_d = nc.dram_tensor("out_h", [2, NCHUNK, 128, TC, KH, BC], F16,
                           kind="ExternalOutput")

    with tile.TileContext(nc) as tc:
        with (
            tc.tile_pool(name="const", bufs=1) as const,
            tc.tile_pool(name="vload", bufs=4) as vload,
            tc.tile_pool(name="vsbp", bufs=1) as vsbp,
            tc.tile_pool(name="hst0", bufs=2) as hst0p,
            tc.tile_pool(name="hst1", bufs=2) as hst1p,
            tc.tile_pool(name="sg", bufs=3) as sgp,
            tc.tile_pool(name="cst", bufs=2) as cstp,
            tc.tile_pool(name="h8p", bufs=3) as h8p,
            tc.tile_pool(name="tmp", bufs=2) as tmpp,
            tc.tile_pool(name="psv", bufs=2, space="PSUM") as psv,
            tc.tile_pool(name="pg0", bufs=3, space="PSUM") as pg0p,
            tc.tile_pool(name="pg1", bufs=3, space="PSUM") as pg1p,
        ):
            # resident weights; order matters: wet + first video chunks gate
            # the embed prologue, wih/whh only gate round 0. wet is loaded as
            # KP column slices, wih/whh per-direction, so nothing waits on one
            # monolithic transfer.
            wet = const.tile([128, KP, KF, 128], F16)
            wih = const.tile([128, 2, KPB, 4 * H], F16)
            whh = const.tile([128, 2, KH, 4 * H], F8)

            # constant one-hot input column for the bias matmul: partition 0
            # carries 1.0 so (bias-row weight tile) @ onec adds the gate bias
            onec = const.tile([128, BC], F16)
            nc.gpsimd.memset(onec[:], 0.0)
            nc.gpsimd.memset(onec[0:1, :], 1.0)

            # full embed output, shared by both directions
            vsb = vsbp.tile([128, KP, T, BC], F16)

            pgp = (pg0p, pg1p)

            vch_tiles = {}

            def dma_chunk(c):
                vch = vload.tile([128, KF, TC * BC], F16, tag="vch")
                nc.sync.dma_start(vch[:], vt_d.ap()[c].rearrange("p ko t b -> p ko (t b)"))
                vch_tiles[c] = vch

            def embed_item(c, mp):
                vch = vch_tiles[c]
                pv = psv.tile([128, TC * BC], F32, tag="pv")
                for ko in range(KF):
                    nc.tensor.matmul(
                        pv[:],
                        wet[:, mp, ko, :],
                        vch[:, ko, :],
                        start=(ko == 0),
                        stop=(ko == KF - 1),
                    )
                # psum -> sbuf copy on DVE (ACT is the scarcer engine here)
                nc.vector.tensor_scalar_mul(
                    vsb[:, mp, c * TC : (c + 1) * TC, :].rearrange("p t b -> p (t b)"),
                    pv[:],
                    1.0,
                )

            def xg_mms(d, t):
                """Input-projection + bias matmuls for dir d, step t (no h
                dependence). The pg tile is one PSUM bank = one zero region:
                start=True only on the very first matmul into the tile (it
                zeroes the whole region), stop=True only on the very last
                (end of hh, or end of xg at t==0)."""
                v_idx = t if d == 0 else T - 1 - t
                pg = pgp[d].tile([128, 4, KH, BC], F32, tag=f"pg{d}")
                last = t == 0  # step 0 has no hh matmuls: close the group now
                for m in range(MG):
                    kind, kh = divmod(m, KH)
                    for kp in range(KPB):
                        rhs = onec[:] if kp == KP else vsb[:, kp, v_idx, :]
                        nc.tensor.matmul(
                            pg[:, kind, kh, :],
                            wih[:, d, kp, ts(m, 128)],
                            rhs,
                            start=(m == 0 and kp == 0),
                            stop=(last and m == MG - 1 and kp == KPB - 1),
                            skip_group_check=True,
                        )
                return pg

            def hh_mms(d, t, pg, h_prev):
                # g tiles first (tanh_g fires early), o last (only h8 needs it)
                for kind in (3, 0, 1, 2):
                  for kh in range(KH):
                    m = kind * KH + kh
                    for q in range(KH // 2):
                        nc.tensor.matmul(
                            pg[:, kind, kh, :],
                            whh[:, d, 2 * q : 2 * q + 2, ts(m, 128)],
                            h_prev[:, 2 * q : 2 * q + 2, :],
                            start=False,
                            stop=(kind == 2 and kh == KH - 1 and q == KH // 2 - 1),
                            perf_mode=mybir.MatmulPerfMode.DoubleRow,
                            skip_group_check=True,
                        )

            def act_gates(d, t, pg):
                sg = sgp.tile([128, 4, KH, BC], F16, tag=f"sg{d}")
                nc.scalar.activation(sg[:, 3, :, :], pg[:, 3, :, :], AF.Tanh,
                                     scale=1.0 / WS)
                nc.scalar.activation(sg[:, 0:2, :, :], pg[:, 0:2, :, :], AF.Sigmoid,
                                     scale=1.0 / WS)
                nc.scalar.activation(sg[:, 2, :, :], pg[:, 2, :, :], AF.Sigmoid,
                                     scale=1.0 / WS)
                return sg

            def dve_c(d, t, sg, c_prev):
                c_new = cstp.tile([128, KH, BC], F32, tag=f"c{d}")
                if t == 0:
                    nc.vector.tensor_tensor(c_new[:], sg[:, 0, :, :], sg[:, 3, :, :],
                                            OP.mult)
                else:
                    m2 = tmpp.tile([128, KH, BC], F16, tag=f"m2{d}")
                    nc.vector.tensor_tensor(m2[:], sg[:, 0, :, :], sg[:, 3, :, :],
                                            OP.mult)
                    m1 = tmpp.tile([128, KH, BC], F32, tag=f"m1{d}")
                    nc.vector.tensor_tensor(m1[:], sg[:, 1, :, :], c_prev[:], OP.mult)
                    nc.vector.tensor_tensor(c_new[:], m1[:], m2[:], OP.add)
                return c_new

            def act_tanh_c(d, c_new):
                tcn = tmpp.tile([128, KH, BC], F16, tag=f"tc{d}")
                nc.scalar.activation(tcn[:], c_new[:], AF.Tanh)
                return tcn

            def dve_h(d, t, sg, tcn, hst):
                # fp8 h feeds the recurrence (critical path); the fp16 output
                # copy is independent and runs whenever DVE has slack
                h8 = h8p.tile([128, KH, BC], F8, tag=f"h8{d}", name="h8")
                nc.vector.tensor_tensor(h8[:], sg[:, 2, :, :], tcn[:], OP.mult)
                h_new = hst[:, t % TC, :, :]
                nc.vector.tensor_tensor(h_new, sg[:, 2, :, :], tcn[:], OP.mult)
                return h8

            # ---- embed work-list construction -------------------------------
            # pair p = chunks CHUNK_ORDER[2p], CHUNK_ORDER[2p+1]; pair 0 runs
            # in the prologue; pair p>=1 must complete before round 8p-1 ends
            # (xg(8p) consumes it), rationed over rounds 8(p-1)..8p-2.
            embed_sched = {r: [] for r in range(T)}
            for p in range(1, 4):
                ca, cb = CHUNK_ORDER[2 * p], CHUNK_ORDER[2 * p + 1]
                items = [(ca, mp) for mp in range(KP)] + [(cb, mp) for mp in range(KP)]
                r0 = 8 * (p - 1)
                # DMA the pair's chunks right away at the window start
                embed_sched[r0].append(("dma", ca))
                embed_sched[r0].append(("dma", cb))
                for j, it in enumerate(items):
                    embed_sched[r0 + j % 7].append(("embed",) + it)

            # ---- prologue ---------------------------------------------------
            nc.sync.dma_start(wet[:, 0], wet_d.ap()[0])
            dma_chunk(0)
            nc.sync.dma_start(wih[:, 0], wih_d.ap()[0])
            for mp in range(1, KP):
                nc.sync.dma_start(wet[:, mp], wet_d.ap()[mp])
            dma_chunk(7)
            nc.sync.dma_start(wih[:, 1], wih_d.ap()[1])
            for d in (0, 1):
                nc.sync.dma_start(whh[:, d], whh_d.ap()[d])
            for mp in range(KP):
                embed_item(0, mp)
            for mp in range(KP):
                embed_item(7, mp)
            pg_cur = [xg_mms(0, 0), xg_mms(1, 0)]

            h_prev = [None, None]
            c_prev = [None, None]
            hst = [None, None]

            # ---- scan rounds ------------------------------------------------
            for t in range(T):
                if t % TC == 0:
                    hst[0] = hst0p.tile([128, TC, KH, BC], F16, tag="hst0", name="hst0")
                    hst[1] = hst1p.tile([128, TC, KH, BC], F16, tag="hst1", name="hst1")

                # PE: finish this round's gates (hh), per dir
                for d in (0, 1):
                    if t > 0:
                        hh_mms(d, t, pg_cur[d], h_prev[d])

                # ACT: gate activations (tanh_c comes later, after DVE c)
                sgs = [act_gates(d, t, pg_cur[d]) for d in (0, 1)]
                # DVE: c chain per dir
                c_news = [dve_c(d, t, sgs[d], c_prev[d]) for d in (0, 1)]
                # ACT: tanh(c)
                tcns = [act_tanh_c(d, c_news[d]) for d in (0, 1)]
                # DVE: h = s_o * tanh(c)
                for d in (0, 1):
                    h_prev[d] = dve_h(d, t, sgs[d], tcns[d], hst[d])
                    c_prev[d] = c_news[d]

                # PE filler: next round's input projections (h-independent)
                if t + 1 < T:
                    pg_cur = [xg_mms(0, t + 1), xg_mms(1, t + 1)]

                # rationed embed work + chunk DMAs
                for item in embed_sched[t]:
                    if item[0] == "dma":
                        dma_chunk(item[1])
                    else:
                        embed_item(item[1], item[2])

                # chunk boundary: ship outputs
                if t % TC == TC - 1:
                    c_out = t // TC
                    nc.gpsimd.dma_start(out_d.ap()[0][c_out], hst[0][:])
                    nc.gpsimd.dma_start(out_d.ap()[1][c_out], hst[1][:])

    nc.compile()
    return nc


_CACHED_NC = None


def _get_nc():
    global _CACHED_NC
    if _CACHED_NC is None:
        _CACHED_NC = build_nc()
    return _CACHED_NC


def _prep_inputs(video_feats, W_e, b_e, W_ih1, W_hh1, b_ih1, b_hh1,
                 W_ih2, W_hh2, b_ih2, b_hh2):
    """Build the 8 per-core input maps (host-side shard + layout prep)."""
    # [KP, 128, KF, 128]: column-slice mp, partition, F tile, column
    wet = np.ascontiguousarray(
        W_e.T.astype(np.float16).reshape(KF, 128, KP, 128).transpose(2, 1, 0, 3)
    )

    # gate kind order [i, f, o, g] so one sigmoid op covers kinds 0..2
    perm = np.concatenate([
        np.arange(0 * H, 1 * H),   # i
        np.arange(1 * H, 2 * H),   # f
        np.arange(3 * H, 4 * H),   # o
        np.arange(2 * H, 3 * H),   # g
    ])
    wih_all = np.zeros((2, 128, KPB, 4 * H), np.float16)
    whh_all = np.zeros((2, 128, KH, 4 * H), mybir.dt.np(F8))
    for d, (W_ih, W_hh, b_ih, b_hh) in enumerate((
        (W_ih1, W_hh1, b_ih1, b_hh1),
        (W_ih2, W_hh2, b_ih2, b_hh2),
    )):
        wih_all[d, :, :KP, :] = (
            (W_ih[perm].T * WS).astype(np.float16)
            .reshape(KP, 128, 4 * H).transpose(1, 0, 2)
        )
        # gate bias (incl. embed bias pushed through W_ih) rides contraction
        # row 0 of the extra input tile
        b_full = ((b_ih + b_hh + W_ih @ b_e)[perm] * WS).astype(np.float16)
        wih_all[d, 0, KP, :] = b_full
        whh_all[d, :, :, :] = (
            (W_hh[perm].T * WS).astype(mybir.dt.np(F8))
            .reshape(KH, 128, 4 * H).transpose(1, 0, 2)
        )

    vt_full = np.ascontiguousarray(video_feats.transpose(2, 1, 0)).astype(np.float16)

    in_maps = []
    for core in range(8):
        src = vt_full[:, :, core * BC : (core + 1) * BC]
        vt = np.ascontiguousarray(
            src.reshape(KF, 128, NCHUNK, TC, BC).transpose(2, 1, 0, 3, 4)
        )
        in_maps.append({
            "vt": vt,
            "w_et": wet,
            "w_ih": wih_all,
            "w_hh": whh_all,
        })
    return in_maps


last_exec_ns = None
last_wall_s = None


def kernel(**inputs):
    global last_exec_ns, last_wall_s
    nc = _get_nc()
    inputs = {k: np.asarray(v, dtype=np.float32) for k, v in inputs.items()}
    in_maps = _prep_inputs(**inputs)
    t0 = time.perf_counter()
    res = run_bass_kernel_spmd(nc, in_maps, core_ids=list(range(8)))
    last_wall_s = time.perf_counter() - t0
    last_exec_ns = res.exec_time_ns

    lstm1 = np.empty((B, T, H), np.float32)
    lstm2 = np.empty((B, T, H), np.float32)
    for core in range(8):
        oh = res.results[core]["out_h"]  # [2, NCHUNK, 128, TC, KH, BC] f16
        # [dir, chunk, p, tc, kh, bc] -> [dir, bc, chunk*tc, kh*p]
        h = np.transpose(oh.astype(np.float32), (0, 5, 1, 3, 4, 2)).reshape(
            2, BC, T, H
        )
        lstm1[core * BC : (core + 1) * BC] = h[0]
        lstm2[core * BC : (core + 1) * BC] = h[1][:, ::-1, :]
    return (lstm1, lstm2)
